# revision 21
# baseline (speedup 1.0000x reference)
"""DigitCaps dynamic-routing kernel for 8 Trainium2 NeuronCores (v3).

Problem: x(32,16384,8) f32, W(10,16384,8,16) f32 -> v(32,10,16) f32
  u_hat[b,j,p,o] = sum_d x[b,p,d] W[j,p,d,o]   (never materialized)
  3 routing iterations (softmax over j, weighted sums over p).

Shard P=16384 over 8 cores (P_loc=2048 = 16 tiles of 128). All weights
bf16, resident in SBUF (no per-iteration W streaming). Per routing step:

  s-phase (PE): s[b,j,o] = sum_{p,d} y * W with y = e * x~ (bf16).
    it0: c==0.1 -> single psum group, lhsT=x [p^,32], rhs=W [p^,160].
    it>0: j-quad packing: lhsT = y[p^,(j4,b)=128], rhs = W[p^,(j4,o)=64];
    out[(j4,b),(j4,o)] - only the j-diagonal [32b,16o] blocks are used;
    3 quad groups (j 0-3, 4-7, 8-9) accumulate over all (t,d) in psum.
    Extraction: 10 cross-partition-base scalar copies psum->sbuf.
  AllReduce s (20KB) -> squash -> v (bf16). Junk matmuls keep the PE's
    HAM activity monitor warm (2.4 GHz) across the AllReduce window.
  v transposed via PE -> DRAM [160,32] -> 8 diagonal DMAs build the
    block-diag vblk[(d,o), j, (d,b)] moving operand.
  z-phase (PE): z[p^,(d,b)] = wz[(do),p^]^T @ vblk (N=256 per (j,t)).
  consume: uv = sum_d x*z, bb += uv. DVE broadcast ops are ~5x slower
    than contiguous ones, so all operand "broadcasts" (e over d, x~ over
    j, rec over d) are done by the otherwise-idle DMA engines with
    stride-0 reads; every DVE/GpSimd multiply is contiguous bf16.
    Chunks of (j, 4 t's): DVE mult direct from psum (1x), or scalar
    drains psum->sbuf bf16 then DVE (2x) / GpSimd mult; the d-add-tree
    runs u1 per chunk, then batched u2/u3/bb per t-group.

Final iteration outputs the per-core partial s3p[b, j*16+o]; the host
sums partials in f64 and applies the last squash.
"""
import numpy as np
import ml_dtypes
from functools import lru_cache

import concourse.bacc as bacc
import concourse.mybir as mybir
from concourse import tile
from concourse.bass_utils import run_bass_kernel_spmd

F32 = mybir.dt.float32
BF16 = mybir.dt.bfloat16
AX = mybir.AxisListType
ALU = mybir.AluOpType
ACTF = mybir.ActivationFunctionType

B, J, P, D, O = 32, 10, 16384, 8, 16
NCORES = 8
PL = P // NCORES          # 2048
T = PL // 128             # 16 tiles of 128 p's
TG = 4                    # t-group size for z/consume chunks
NTG = T // TG             # 4
JO = J * O                # 160
QUADS = ((0, 4), (4, 4), (8, 2))   # (j0, nj) quad groups for s-phase

DIRECT_J = set()           # DVE mult straight from psum (1x)
SCGP_J = {2, 7}            # scalar drain + gpsimd mult
U1GP_J = {0, 2, 4, 5, 7, 9}  # u1 tree level on gpsimd
YGP_T = set()              # y-mult tiles on gpsimd (DVE contiguous is 4x faster)
NJUNK = 120                # HAM-warming junk matmuls per AllReduce window


def _emit(nc, n_cores):
    xb_d = nc.dram_tensor("xb", [128, T, D, B], BF16, kind="ExternalInput")
    xe_d = nc.dram_tensor("xe", [128, T, D, J, B], BF16,
                          kind="ExternalInput")
    ws_d = nc.dram_tensor("ws", [128, T, D, JO], BF16, kind="ExternalInput")
    wz_d = nc.dram_tensor("wz", [128, J, T, 128], BF16, kind="ExternalInput")
    id_d = nc.dram_tensor("ident", [32, 32], BF16, kind="ExternalInput")
    s3p = nc.dram_tensor("s3p", [B, JO], F32, kind="ExternalOutput")

    with tile.TileContext(nc) as tc:
        with (
            tc.tile_pool(name="per", bufs=1) as per,
            tc.tile_pool(name="yp", bufs=2) as yp,
            tc.tile_pool(name="cep", bufs=2) as cep,
            tc.tile_pool(name="xep", bufs=2) as xep,
            tc.tile_pool(name="ctp", bufs=2) as ctp,
            tc.tile_pool(name="hap", bufs=2) as hap,
            tc.tile_pool(name="zdp", bufs=2) as zdp,
            tc.tile_pool(name="u1p", bufs=1) as u1p,
            tc.tile_pool(name="u2p", bufs=1) as u2p,
            tc.tile_pool(name="u3p", bufs=1) as u3p,
            tc.tile_pool(name="small", bufs=4) as small,
            tc.tile_pool(name="sps", bufs=1, space="PSUM") as sps,
            tc.tile_pool(name="vtp", bufs=1, space="PSUM") as vtp,
            tc.tile_pool(name="zps", bufs=2, space="PSUM") as zps,
            tc.tile_pool(name="dram", bufs=4, space="DRAM") as dramp,
        ):
            # warmup collective: absorbs ncfw's first-collective barrier
            wu_in = dramp.tile([B, 16], F32)
            wu_out = dramp.tile([B, 16], F32)
            wu_sb = small.tile([B, 16], F32)
            nc.vector.memset(wu_sb[:], 0.0)
            nc.scalar.dma_start(wu_in[:], wu_sb[:])
            nc.gpsimd.collective_compute(
                "AllReduce", ALU.add,
                replica_groups=[list(range(n_cores))],
                ins=[wu_in[:].opt()], outs=[wu_out[:].opt()],
            )

            xb = per.tile([128, T, D, B], BF16)
            nc.sync.dma_start(xb[:], xb_d[:, :, :, :])
            ws = per.tile([128, T, D, JO], BF16)
            for t in range(T):
                nc.sync.dma_start(ws[:, t, :, :], ws_d[:, t, :, :])
            wz = per.tile([128, J, T, 128], BF16)
            nc.sync.dma_start(wz[:], wz_d[:, :, :, :])
            ident = per.tile([32, 32], BF16)
            nc.sync.dma_start(ident[:], id_d[:, :])

            bb = per.tile([128, T, J, B], F32)      # routing logits
            e_sb = per.tile([128, T, J, B], BF16)   # exp(bb)
            se = per.tile([128, T, B], F32)         # sum_j exp
            rec = per.tile([128, T, B], BF16)       # 1/sum
            vblk = per.tile([128, J, D * B], BF16)  # block-diag v
            nc.vector.memset(vblk[:], 0.0)

            # ---------------- it0 s-phase: c == 0.1 ----------------
            s_ps = sps.tile([32, JO], F32, name="sq0")
            for t in range(T):
                for d in range(D):
                    nc.tensor.matmul(
                        s_ps[:, :], xb[:, t, d, :], ws[:, t, d, :],
                        start=(t == 0 and d == 0),
                        stop=(t == T - 1 and d == D - 1))
            sx = small.tile([B, JO], F32)
            nc.scalar.activation(sx[:], s_ps[:, :], ACTF.Copy, scale=0.1)

            def quad_psums():
                return [sps.tile([128, nj * O], F32, name=f"sq{q}")
                        for q, (_, nj) in enumerate(QUADS)]

            def s_extract(qps, dst):
                for j in range(J):
                    q, jr = (0, j) if j < 4 else ((1, j - 4) if j < 8
                                                  else (2, j - 8))
                    nc.scalar.copy(
                        dst[0:B, j * O:(j + 1) * O],
                        qps[q][32 * jr:32 * jr + 32, O * jr:O * jr + O])

            def y_and_smm(qps, t, t0, tg):
                # c = e * (1/se): rec j-broadcast via DMA, then a
                # contiguous 2x multiply; ce = c d-broadcast (DMA).
                c_t = ctp.tile([128, J, B], BF16)
                nc.vector.tensor_mul(
                    c_t[:], e_sb[:, t, :, :],
                    rec[:, t, None, :].broadcast_to([128, J, B]))
                ce = cep.tile([128, D, J, B], BF16)
                nc.sync.dma_start(
                    ce[:], c_t[:, None, :, :].broadcast_to([128, D, J, B]))
                xe = xep.tile([128, D, J, B], BF16)
                nc.sync.dma_start(xe[:], xe_d[:, t, :, :, :])
                y_t = yp.tile([128, D, J, B], BF16)
                yeng = nc.gpsimd if t in YGP_T else nc.vector
                yeng.tensor_mul(y_t[:], xe[:], ce[:])
                for d in range(D):
                    for q, (j0, nj) in enumerate(QUADS):
                        nc.tensor.matmul(
                            qps[q][0:nj * 32, :],
                            y_t[:, d, j0:j0 + nj, :],
                            ws[:, t, d, j0 * O:(j0 + nj) * O],
                            start=(t == t0 and tg == 0 and d == 0),
                            stop=(t == t0 + TG - 1 and tg == NTG - 1
                                  and d == D - 1))

            for k in range(2):      # routing steps that need v (it0, it1)
                # -------- AllReduce s --------
                cc_in = dramp.tile([B, JO], F32)
                cc_out = dramp.tile([B, JO], F32)
                nc.scalar.dma_start(cc_in[:], sx[:])
                # junk matmuls span the AllReduce window so the PE's HAM
                # clock gate stays at 8/8 for the z-phase that follows.
                jnk_ps = vtp.tile([1, 512], F32, name="vt_ps")
                jrhs = ws[:, 0, :, :].rearrange("p d jo -> p (d jo)")
                for i in range(NJUNK):
                    nc.tensor.matmul(
                        jnk_ps[:, :], ws[:, 0, 0, 0:1], jrhs[:, 0:512],
                        start=True, stop=True)
                nc.gpsimd.collective_compute(
                    "AllReduce", ALU.add,
                    replica_groups=[list(range(n_cores))],
                    ins=[cc_in[:].opt()], outs=[cc_out[:].opt()],
                )
                s_f = small.tile([B, JO], F32)
                nc.scalar.dma_start(s_f[:], cc_out[:])

                # -------- squash -> v (bf16) --------
                t2 = small.tile([B, JO], F32)
                nc.vector.tensor_mul(t2[:], s_f[:], s_f[:])
                sq = small.tile([B, J], F32)
                nc.vector.tensor_reduce(
                    sq[:, :, None], t2.rearrange("b (j o) -> b j o", j=J),
                    AX.X, ALU.add)
                r_ = small.tile([B, J], F32)
                nc.scalar.activation(r_[:], sq[:], ACTF.Sqrt)
                den = small.tile([B, J], F32)
                nc.vector.scalar_tensor_tensor(
                    den[:], sq[:], 1.0, r_[:], ALU.add, ALU.mult)
                rc2 = small.tile([B, J], F32)
                nc.vector.reciprocal(rc2[:], den[:])
                f_ = small.tile([B, J], F32)
                nc.vector.tensor_mul(f_[:], sq[:], rc2[:])
                v_sb = small.tile([B, JO], BF16)
                nc.vector.tensor_mul(
                    v_sb.rearrange("b (j o) -> b j o", j=J),
                    s_f.rearrange("b (j o) -> b j o", j=J),
                    f_[:, :, None].broadcast_to([B, J, O]))

                # -------- v -> vT (PE transpose) -> DRAM -> vblk --------
                vt1_ps = vtp.tile([128, 32], BF16, name="vt_ps")
                nc.tensor.transpose(vt1_ps[:], v_sb[:, 0:128], ident[:])
                vt1 = small.tile([128, 32], BF16)
                nc.scalar.copy(vt1[:], vt1_ps[:])
                vt2_ps = vtp.tile([32, 32], BF16, name="vt_ps")
                nc.tensor.transpose(vt2_ps[:], v_sb[:, 128:160], ident[:])
                vt2 = small.tile([32, 32], BF16)
                nc.scalar.copy(vt2[:], vt2_ps[:])
                vt_dr = dramp.tile([JO, B], BF16)
                nc.scalar.dma_start(vt_dr[0:128, :], vt1[:])
                nc.scalar.dma_start(vt_dr[128:160, :], vt2[:])
                vt_v = vt_dr.rearrange("(j o) b -> o j b", j=J)
                for d in range(D):
                    nc.scalar.dma_start(
                        vblk[d * O:(d + 1) * O, :, d * B:(d + 1) * B],
                        vt_v[:, :, :])

                # -------- z-phase + consume + (softmax/y/s of k+1) ------
                qps = quad_psums()
                for tg in range(NTG):
                    t0 = tg * TG
                    u1t = u1p.tile([128, J, TG, 4, B], BF16)
                    for j in range(J):
                        z_ps = zps.tile([128, TG, D * B], F32)
                        for t4 in range(TG):
                            nc.tensor.matmul(
                                z_ps[:, t4, :], wz[:, j, t0 + t4, :],
                                vblk[:, j, :],
                                start=(t4 % 2 == 0), stop=(t4 % 2 == 1))
                        xs = xb[:, t0:t0 + TG, :, :].rearrange(
                            "p t d b -> p (t d b)")
                        ha = hap.tile([128, TG * D * B], BF16)
                        zv = z_ps.rearrange("p t db -> p (t db)")
                        if j in DIRECT_J:
                            nc.vector.tensor_mul(ha[:], zv, xs)
                        else:
                            zd = zdp.tile([128, TG * D * B], BF16)
                            nc.scalar.copy(zd[:], zv)
                            eng = nc.gpsimd if j in SCGP_J else nc.vector
                            eng.tensor_mul(ha[:], zd[:], xs)
                        hv = ha.rearrange("p (t d b) -> p t d b", t=TG, d=D)
                        ueng = nc.gpsimd if j in U1GP_J else nc.vector
                        ueng.tensor_add(
                            u1t[:, j, :, :, :], hv[:, :, 0:4, :],
                            hv[:, :, 4:8, :])
                    # batched u2/u3/bb for the whole t-group
                    u2t = u2p.tile([128, J, TG, 2, B], BF16)
                    nc.vector.tensor_add(
                        u2t[:], u1t[:, :, :, 0:2, :], u1t[:, :, :, 2:4, :])
                    u3t = u3p.tile([128, J, TG, B], BF16)
                    nc.vector.tensor_add(
                        u3t[:], u2t[:, :, :, 0, :], u2t[:, :, :, 1, :])
                    u3v = u3t.rearrange("p j t b -> p t j b")
                    bb_sl = bb[:, t0:t0 + TG, :, :]
                    if k == 0:
                        nc.vector.tensor_copy(bb_sl, u3v)
                    else:
                        nc.vector.tensor_add(bb_sl, bb_sl, u3v)

                    # ---- softmax for this t-group (bb complete) ----
                    nc.scalar.activation(
                        e_sb[:, t0:t0 + TG, :, :], bb[:, t0:t0 + TG, :, :],
                        ACTF.Exp)
                    nc.vector.tensor_reduce(
                        se[:, t0:t0 + TG, :, None],
                        e_sb[:, t0:t0 + TG, :, :].rearrange(
                            "p t j b -> p t b j"),
                        AX.X, ALU.add)
                    with nc.allow_low_precision(
                            reason="1/se as bf16 feeds bf16 c; validated"):
                        nc.vector.reciprocal(
                            rec[:, t0:t0 + TG, :], se[:, t0:t0 + TG, :])
                    # ---- y + s-matmuls of step k+1 for this t-group ----
                    for t in range(t0, t0 + TG):
                        y_and_smm(qps, t, t0, tg)
                sx = small.tile([B, JO], F32)
                s_extract(qps, sx)

            nc.sync.dma_start(s3p[:, :], sx[:])
    return nc


@lru_cache(maxsize=2)
def _build(n_cores):
    nc = bacc.Bacc("TRN2", target_bir_lowering=False, debug=False,
                   num_devices=n_cores)
    _emit(nc, n_cores)
    nc.compile()
    return nc


def _prep_inputs(x, W):
    """Host-side shard + relayout. Returns list of per-core input dicts."""
    x = np.asarray(x, dtype=np.float32)
    W = np.asarray(W, dtype=np.float32)
    ident = np.eye(32, dtype=ml_dtypes.bfloat16)
    in_maps = []
    for c in range(NCORES):
        xc = x[:, c * PL:(c + 1) * PL, :]              # (B, PL, D)
        Wc = W[:, c * PL:(c + 1) * PL, :, :]           # (J, PL, D, O)
        xr = np.ascontiguousarray(
            xc.reshape(B, T, 128, D).transpose(2, 1, 3, 0))        # [128,T,D,B]
        wsr = np.ascontiguousarray(
            Wc.reshape(J, T, 128, D, O).transpose(2, 1, 3, 0, 4)
            .reshape(128, T, D, JO))                               # [128,T,D,JO]
        wzr = np.ascontiguousarray(
            Wc.reshape(J, T, 128, D, O).transpose(3, 4, 0, 1, 2)
            .reshape(128, J, T, 128))                              # [(d,o),J,T,p]
        xrb = xr.astype(ml_dtypes.bfloat16)
        in_maps.append({
            "xb": xrb,
            "xe": np.ascontiguousarray(
                np.broadcast_to(xrb[:, :, :, None, :],
                                (128, T, D, J, B))),
            "ws": wsr.astype(ml_dtypes.bfloat16),
            "wz": wzr.astype(ml_dtypes.bfloat16),
            "ident": ident,
        })
    return in_maps


def _squash_np(s):
    sq = np.sum(s * s, axis=-1, keepdims=True)
    return s * (sq / ((1.0 + sq) * np.sqrt(sq)))


def kernel(x, W):
    nc = _build(NCORES)
    in_maps = _prep_inputs(x, W)
    res = run_bass_kernel_spmd(nc, in_maps, list(range(NCORES)))
    s3 = np.zeros((B, JO), np.float64)
    for r in res.results:
        s3 += r["s3p"].astype(np.float64)
    v = _squash_np(s3.reshape(B, J, O))
    return v.astype(np.float32)


# revision 22
# speedup vs baseline: 1.0109x; 1.0109x over previous
"""DigitCaps dynamic-routing kernel for 8 Trainium2 NeuronCores (v3).

Problem: x(32,16384,8) f32, W(10,16384,8,16) f32 -> v(32,10,16) f32
  u_hat[b,j,p,o] = sum_d x[b,p,d] W[j,p,d,o]   (never materialized)
  3 routing iterations (softmax over j, weighted sums over p).

Shard P=16384 over 8 cores (P_loc=2048 = 16 tiles of 128). All weights
bf16, resident in SBUF (no per-iteration W streaming). Per routing step:

  s-phase (PE): s[b,j,o] = sum_{p,d} y * W with y = e * x~ (bf16).
    it0: c==0.1 -> single psum group, lhsT=x [p^,32], rhs=W [p^,160].
    it>0: j-quad packing: lhsT = y[p^,(j4,b)=128], rhs = W[p^,(j4,o)=64];
    out[(j4,b),(j4,o)] - only the j-diagonal [32b,16o] blocks are used;
    3 quad groups (j 0-3, 4-7, 8-9) accumulate over all (t,d) in psum.
    Extraction: 10 cross-partition-base scalar copies psum->sbuf.
  AllReduce s (20KB) -> squash -> v (bf16). Junk matmuls keep the PE's
    HAM activity monitor warm (2.4 GHz) across the AllReduce window.
  v transposed via PE -> DRAM [160,32] -> 8 diagonal DMAs build the
    block-diag vblk[(d,o), j, (d,b)] moving operand.
  z-phase (PE): z[p^,(d,b)] = wz[(do),p^]^T @ vblk (N=256 per (j,t)).
  consume: uv = sum_d x*z, bb += uv. DVE broadcast ops are ~5x slower
    than contiguous ones, so all operand "broadcasts" (e over d, x~ over
    j, rec over d) are done by the otherwise-idle DMA engines with
    stride-0 reads; every DVE/GpSimd multiply is contiguous bf16.
    Chunks of (j, 4 t's): DVE mult direct from psum (1x), or scalar
    drains psum->sbuf bf16 then DVE (2x) / GpSimd mult; the d-add-tree
    runs u1 per chunk, then batched u2/u3/bb per t-group.

Final iteration outputs the per-core partial s3p[b, j*16+o]; the host
sums partials in f64 and applies the last squash.
"""
import numpy as np
import ml_dtypes
from functools import lru_cache

import concourse.bacc as bacc
import concourse.mybir as mybir
from concourse import tile
from concourse.bass_utils import run_bass_kernel_spmd

F32 = mybir.dt.float32
BF16 = mybir.dt.bfloat16
AX = mybir.AxisListType
ALU = mybir.AluOpType
ACTF = mybir.ActivationFunctionType

B, J, P, D, O = 32, 10, 16384, 8, 16
NCORES = 8
PL = P // NCORES          # 2048
T = PL // 128             # 16 tiles of 128 p's
TG = 4                    # t-group size for z/consume chunks
NTG = T // TG             # 4
JO = J * O                # 160
QUADS = ((0, 4), (4, 4), (8, 2))   # (j0, nj) quad groups for s-phase

DIRECT_J = {0, 5}          # DVE mult straight from psum (1x)
SCGP_J = {2, 7}            # scalar drain + gpsimd mult
U1GP_J = {2, 4, 7, 9}      # u1 tree level on gpsimd
YGP_T = set()              # y-mult tiles on gpsimd (DVE contiguous is 4x faster)
NJUNK = 120                # HAM-warming junk matmuls per AllReduce window


def _emit(nc, n_cores):
    xb_d = nc.dram_tensor("xb", [128, T, D, B], BF16, kind="ExternalInput")
    xe_d = nc.dram_tensor("xe", [128, T, D, J, B], BF16,
                          kind="ExternalInput")
    ws_d = nc.dram_tensor("ws", [128, T, D, JO], BF16, kind="ExternalInput")
    wz_d = nc.dram_tensor("wz", [128, J, T, 128], BF16, kind="ExternalInput")
    id_d = nc.dram_tensor("ident", [32, 32], BF16, kind="ExternalInput")
    s3p = nc.dram_tensor("s3p", [B, JO], F32, kind="ExternalOutput")

    with tile.TileContext(nc) as tc:
        with (
            tc.tile_pool(name="per", bufs=1) as per,
            tc.tile_pool(name="yp", bufs=2) as yp,
            tc.tile_pool(name="cep", bufs=2) as cep,
            tc.tile_pool(name="xep", bufs=2) as xep,
            tc.tile_pool(name="ctp", bufs=2) as ctp,
            tc.tile_pool(name="hap", bufs=2) as hap,
            tc.tile_pool(name="zdp", bufs=2) as zdp,
            tc.tile_pool(name="u1p", bufs=1) as u1p,
            tc.tile_pool(name="u2p", bufs=1) as u2p,
            tc.tile_pool(name="u3p", bufs=1) as u3p,
            tc.tile_pool(name="small", bufs=4) as small,
            tc.tile_pool(name="sps", bufs=1, space="PSUM") as sps,
            tc.tile_pool(name="vtp", bufs=1, space="PSUM") as vtp,
            tc.tile_pool(name="zps", bufs=2, space="PSUM") as zps,
            tc.tile_pool(name="dram", bufs=4, space="DRAM") as dramp,
        ):
            # warmup collective: absorbs ncfw's first-collective barrier
            wu_in = dramp.tile([B, 16], F32)
            wu_out = dramp.tile([B, 16], F32)
            wu_sb = small.tile([B, 16], F32)
            nc.vector.memset(wu_sb[:], 0.0)
            nc.scalar.dma_start(wu_in[:], wu_sb[:])
            nc.gpsimd.collective_compute(
                "AllReduce", ALU.add,
                replica_groups=[list(range(n_cores))],
                ins=[wu_in[:].opt()], outs=[wu_out[:].opt()],
            )

            xb = per.tile([128, T, D, B], BF16)
            nc.sync.dma_start(xb[:], xb_d[:, :, :, :])
            ws = per.tile([128, T, D, JO], BF16)
            for t in range(T):
                nc.sync.dma_start(ws[:, t, :, :], ws_d[:, t, :, :])
            wz = per.tile([128, J, T, 128], BF16)
            nc.sync.dma_start(wz[:], wz_d[:, :, :, :])
            ident = per.tile([32, 32], BF16)
            nc.sync.dma_start(ident[:], id_d[:, :])

            bb = per.tile([128, T, J, B], F32)      # routing logits
            e_sb = per.tile([128, T, J, B], BF16)   # exp(bb)
            se = per.tile([128, T, B], F32)         # sum_j exp
            rec = per.tile([128, T, B], BF16)       # 1/sum
            vblk = per.tile([128, J, D * B], BF16)  # block-diag v
            nc.vector.memset(vblk[:], 0.0)

            # ---------------- it0 s-phase: c == 0.1 ----------------
            s_ps = sps.tile([32, JO], F32, name="sq0")
            for t in range(T):
                for d in range(D):
                    nc.tensor.matmul(
                        s_ps[:, :], xb[:, t, d, :], ws[:, t, d, :],
                        start=(t == 0 and d == 0),
                        stop=(t == T - 1 and d == D - 1))
            sx = small.tile([B, JO], F32)
            nc.scalar.activation(sx[:], s_ps[:, :], ACTF.Copy, scale=0.1)

            def quad_psums():
                return [sps.tile([128, nj * O], F32, name=f"sq{q}")
                        for q, (_, nj) in enumerate(QUADS)]

            def s_extract(qps, dst):
                for j in range(J):
                    q, jr = (0, j) if j < 4 else ((1, j - 4) if j < 8
                                                  else (2, j - 8))
                    nc.scalar.copy(
                        dst[0:B, j * O:(j + 1) * O],
                        qps[q][32 * jr:32 * jr + 32, O * jr:O * jr + O])

            def y_and_smm(qps, t, t0, tg):
                # c = e * (1/se): rec j-broadcast via DMA, then a
                # contiguous 2x multiply; ce = c d-broadcast (DMA).
                c_t = ctp.tile([128, J, B], BF16)
                nc.vector.tensor_mul(
                    c_t[:], e_sb[:, t, :, :],
                    rec[:, t, None, :].broadcast_to([128, J, B]))
                ce = cep.tile([128, D, J, B], BF16)
                nc.sync.dma_start(
                    ce[:], c_t[:, None, :, :].broadcast_to([128, D, J, B]))
                xe = xep.tile([128, D, J, B], BF16)
                nc.sync.dma_start(xe[:], xe_d[:, t, :, :, :])
                y_t = yp.tile([128, D, J, B], BF16)
                yeng = nc.gpsimd if t in YGP_T else nc.vector
                yeng.tensor_mul(y_t[:], xe[:], ce[:])
                for d in range(D):
                    for q, (j0, nj) in enumerate(QUADS):
                        nc.tensor.matmul(
                            qps[q][0:nj * 32, :],
                            y_t[:, d, j0:j0 + nj, :],
                            ws[:, t, d, j0 * O:(j0 + nj) * O],
                            start=(t == t0 and tg == 0 and d == 0),
                            stop=(t == t0 + TG - 1 and tg == NTG - 1
                                  and d == D - 1))

            for k in range(2):      # routing steps that need v (it0, it1)
                # -------- AllReduce s --------
                cc_in = dramp.tile([B, JO], F32)
                cc_out = dramp.tile([B, JO], F32)
                nc.scalar.dma_start(cc_in[:], sx[:])
                # junk matmuls span the AllReduce window so the PE's HAM
                # clock gate stays at 8/8 for the z-phase that follows.
                jnk_ps = vtp.tile([1, 512], F32, name="vt_ps")
                jrhs = ws[:, 0, :, :].rearrange("p d jo -> p (d jo)")
                for i in range(NJUNK):
                    nc.tensor.matmul(
                        jnk_ps[:, :], ws[:, 0, 0, 0:1], jrhs[:, 0:512],
                        start=True, stop=True)
                nc.gpsimd.collective_compute(
                    "AllReduce", ALU.add,
                    replica_groups=[list(range(n_cores))],
                    ins=[cc_in[:].opt()], outs=[cc_out[:].opt()],
                )
                s_f = small.tile([B, JO], F32)
                nc.scalar.dma_start(s_f[:], cc_out[:])

                # -------- squash -> v (bf16) --------
                t2 = small.tile([B, JO], F32)
                nc.vector.tensor_mul(t2[:], s_f[:], s_f[:])
                sq = small.tile([B, J], F32)
                nc.vector.tensor_reduce(
                    sq[:, :, None], t2.rearrange("b (j o) -> b j o", j=J),
                    AX.X, ALU.add)
                r_ = small.tile([B, J], F32)
                nc.scalar.activation(r_[:], sq[:], ACTF.Sqrt)
                den = small.tile([B, J], F32)
                nc.vector.scalar_tensor_tensor(
                    den[:], sq[:], 1.0, r_[:], ALU.add, ALU.mult)
                rc2 = small.tile([B, J], F32)
                nc.vector.reciprocal(rc2[:], den[:])
                f_ = small.tile([B, J], F32)
                nc.vector.tensor_mul(f_[:], sq[:], rc2[:])
                v_sb = small.tile([B, JO], BF16)
                nc.vector.tensor_mul(
                    v_sb.rearrange("b (j o) -> b j o", j=J),
                    s_f.rearrange("b (j o) -> b j o", j=J),
                    f_[:, :, None].broadcast_to([B, J, O]))

                # -------- v -> vT (PE transpose) -> DRAM -> vblk --------
                vt1_ps = vtp.tile([128, 32], BF16, name="vt_ps")
                nc.tensor.transpose(vt1_ps[:], v_sb[:, 0:128], ident[:])
                vt1 = small.tile([128, 32], BF16)
                nc.scalar.copy(vt1[:], vt1_ps[:])
                vt2_ps = vtp.tile([32, 32], BF16, name="vt_ps")
                nc.tensor.transpose(vt2_ps[:], v_sb[:, 128:160], ident[:])
                vt2 = small.tile([32, 32], BF16)
                nc.scalar.copy(vt2[:], vt2_ps[:])
                vt_dr = dramp.tile([JO, B], BF16)
                nc.scalar.dma_start(vt_dr[0:128, :], vt1[:])
                nc.scalar.dma_start(vt_dr[128:160, :], vt2[:])
                vt_v = vt_dr.rearrange("(j o) b -> o j b", j=J)
                for d in range(D):
                    nc.scalar.dma_start(
                        vblk[d * O:(d + 1) * O, :, d * B:(d + 1) * B],
                        vt_v[:, :, :])

                # -------- z-phase + consume + (softmax/y/s of k+1) ------
                qps = quad_psums()
                for tg in range(NTG):
                    t0 = tg * TG
                    u1t = u1p.tile([128, J, TG, 4, B], BF16)
                    for j in range(J):
                        z_ps = zps.tile([128, TG, D * B], F32)
                        for t4 in range(TG):
                            nc.tensor.matmul(
                                z_ps[:, t4, :], wz[:, j, t0 + t4, :],
                                vblk[:, j, :],
                                start=(t4 % 2 == 0), stop=(t4 % 2 == 1))
                        xs = xb[:, t0:t0 + TG, :, :].rearrange(
                            "p t d b -> p (t d b)")
                        ha = hap.tile([128, TG * D * B], BF16)
                        zv = z_ps.rearrange("p t db -> p (t db)")
                        if j in DIRECT_J:
                            nc.vector.tensor_mul(ha[:], zv, xs)
                        else:
                            zd = zdp.tile([128, TG * D * B], BF16)
                            nc.scalar.copy(zd[:], zv)
                            eng = nc.gpsimd if j in SCGP_J else nc.vector
                            eng.tensor_mul(ha[:], zd[:], xs)
                        hv = ha.rearrange("p (t d b) -> p t d b", t=TG, d=D)
                        ueng = nc.gpsimd if j in U1GP_J else nc.vector
                        ueng.tensor_add(
                            u1t[:, j, :, :, :], hv[:, :, 0:4, :],
                            hv[:, :, 4:8, :])
                    # batched u2/u3/bb for the whole t-group
                    u2t = u2p.tile([128, J, TG, 2, B], BF16)
                    nc.vector.tensor_add(
                        u2t[:], u1t[:, :, :, 0:2, :], u1t[:, :, :, 2:4, :])
                    u3t = u3p.tile([128, J, TG, B], BF16)
                    nc.vector.tensor_add(
                        u3t[:], u2t[:, :, :, 0, :], u2t[:, :, :, 1, :])
                    u3v = u3t.rearrange("p j t b -> p t j b")
                    bb_sl = bb[:, t0:t0 + TG, :, :]
                    if k == 0:
                        nc.vector.tensor_copy(bb_sl, u3v)
                    else:
                        nc.vector.tensor_add(bb_sl, bb_sl, u3v)

                    # ---- softmax for this t-group (bb complete) ----
                    nc.scalar.activation(
                        e_sb[:, t0:t0 + TG, :, :], bb[:, t0:t0 + TG, :, :],
                        ACTF.Exp)
                    nc.vector.tensor_reduce(
                        se[:, t0:t0 + TG, :, None],
                        e_sb[:, t0:t0 + TG, :, :].rearrange(
                            "p t j b -> p t b j"),
                        AX.X, ALU.add)
                    with nc.allow_low_precision(
                            reason="1/se as bf16 feeds bf16 c; validated"):
                        nc.vector.reciprocal(
                            rec[:, t0:t0 + TG, :], se[:, t0:t0 + TG, :])
                    # ---- y + s-matmuls of step k+1 for this t-group ----
                    for t in range(t0, t0 + TG):
                        y_and_smm(qps, t, t0, tg)
                sx = small.tile([B, JO], F32)
                s_extract(qps, sx)

            nc.sync.dma_start(s3p[:, :], sx[:])
    return nc


@lru_cache(maxsize=2)
def _build(n_cores):
    nc = bacc.Bacc("TRN2", target_bir_lowering=False, debug=False,
                   num_devices=n_cores)
    _emit(nc, n_cores)
    nc.compile()
    return nc


def _prep_inputs(x, W):
    """Host-side shard + relayout. Returns list of per-core input dicts."""
    x = np.asarray(x, dtype=np.float32)
    W = np.asarray(W, dtype=np.float32)
    ident = np.eye(32, dtype=ml_dtypes.bfloat16)
    in_maps = []
    for c in range(NCORES):
        xc = x[:, c * PL:(c + 1) * PL, :]              # (B, PL, D)
        Wc = W[:, c * PL:(c + 1) * PL, :, :]           # (J, PL, D, O)
        xr = np.ascontiguousarray(
            xc.reshape(B, T, 128, D).transpose(2, 1, 3, 0))        # [128,T,D,B]
        wsr = np.ascontiguousarray(
            Wc.reshape(J, T, 128, D, O).transpose(2, 1, 3, 0, 4)
            .reshape(128, T, D, JO))                               # [128,T,D,JO]
        wzr = np.ascontiguousarray(
            Wc.reshape(J, T, 128, D, O).transpose(3, 4, 0, 1, 2)
            .reshape(128, J, T, 128))                              # [(d,o),J,T,p]
        xrb = xr.astype(ml_dtypes.bfloat16)
        in_maps.append({
            "xb": xrb,
            "xe": np.ascontiguousarray(
                np.broadcast_to(xrb[:, :, :, None, :],
                                (128, T, D, J, B))),
            "ws": wsr.astype(ml_dtypes.bfloat16),
            "wz": wzr.astype(ml_dtypes.bfloat16),
            "ident": ident,
        })
    return in_maps


def _squash_np(s):
    sq = np.sum(s * s, axis=-1, keepdims=True)
    return s * (sq / ((1.0 + sq) * np.sqrt(sq)))


def kernel(x, W):
    nc = _build(NCORES)
    in_maps = _prep_inputs(x, W)
    res = run_bass_kernel_spmd(nc, in_maps, list(range(NCORES)))
    s3 = np.zeros((B, JO), np.float64)
    for r in res.results:
        s3 += r["s3p"].astype(np.float64)
    v = _squash_np(s3.reshape(B, J, O))
    return v.astype(np.float32)
